# revision 65
# baseline (speedup 1.0000x reference)
"""BitNet transformer block on 8 Trainium2 NeuronCores (Bass/Tile).

Sharding: DP2 (batch) x TP4 (Megatron-style, sequence-parallel norms).
Cores 0-3 -> batch 0, cores 4-7 -> batch 1. Within each group of 4:
  - weights arrive pre-ternarized (host preprocessing, like the host-side
    sharding transposes): bf16 {-1,0,+1} plus one per-tensor dequant
    scale; every matmul is an exact integer matmul in bf16 with fp32
    PSUM accumulation,
  - each core owns 512 tokens for LN + act_quant (sequence parallel);
    quantized activations (small exact ints carried as bf16) are
    AllGathered,
  - attention is head-parallel (4 heads/core) in S^T layout: exp with no
    max subtraction (scores are O(1)); P^T feeds O^T = v^T @ P^T directly;
    a ones column appended to v yields the softmax denominator,
  - proj/fc2 are row-parallel: raw integer partial sums ReduceScatter in
    bf16 and are dequantized after the reduce,
  - fc1 produces hidden-major (transposed) output directly, so gelu,
    act_quant, and fc2 all run from SBUF with no DRAM spill and no
    activation transpose.
"""

import sys

for _p in ("/opt/trn_rl_repo",):
    if _p not in sys.path:
        sys.path.append(_p)

import numpy as np

_BASS = {}


def _imports():
    if _BASS:
        return _BASS
    import concourse.bass as bass
    import concourse.bass_isa as bass_isa
    import concourse.mybir as mybir
    import concourse.tile as tile
    from concourse import bacc
    from concourse.bass_utils import run_bass_kernel_spmd
    from concourse.masks import make_identity
    _BASS.update(bass=bass, bass_isa=bass_isa, mybir=mybir, tile=tile,
                 bacc=bacc, run=run_bass_kernel_spmd, mkid=make_identity)
    return _BASS

# ---- problem constants (hardcoded per spec) ----
B, N, C, H = 2, 2048, 1024, 16
HID = 4 * C
NCORES, TP = 8, 4
TOK = N // TP            # 512 tokens per core
TT_LOC = TOK // 128      # 4
TT_ALL = N // 128        # 16
HPC = H // TP            # 4 heads per core
DH = C // H              # 64
CS = C // TP             # 256 channel shard (proj contraction)
HS = HID // TP           # 1024 hidden shard
P = 128
KT = C // P              # 8
EPS = 1e-5
MAGIC = 12582912.0       # 1.5 * 2**23: fp32 round-half-even trick
GELU_MIN = 0.17000043    # |min gelu| bound; per-token absmax = max(max, this)
G4 = [[0, 1, 2, 3], [4, 5, 6, 7]]


def build_kernel(g1_trivial, g2_trivial, debug_outs=()):
    m = _imports()
    bass, bass_isa, mybir, tile, bacc = (m["bass"], m["bass_isa"], m["mybir"],
                                         m["tile"], m["bacc"])
    F32, BF16 = mybir.dt.float32, mybir.dt.bfloat16
    AX, ALU, ACTF = (mybir.AxisListType, mybir.AluOpType,
                     mybir.ActivationFunctionType)

    make_identity = m["mkid"]
    nc = bacc.Bacc("TRN2", target_bir_lowering=False, debug=False,
                   num_devices=NCORES)

    x_sh = nc.dram_tensor("x_sh", [TOK, C], F32, kind="ExternalInput")
    # pre-ternarized weights (bf16 {-1,0,1}), pre-transposed for lhsT use
    wqkT_q = nc.dram_tensor("wqkT_q", [C, 2 * CS], BF16, kind="ExternalInput")
    wvT_q = nc.dram_tensor("wvT_q", [C, CS], BF16, kind="ExternalInput")
    wpT_q = nc.dram_tensor("wpT_q", [C, C], BF16, kind="ExternalInput")
    wf1T_q = nc.dram_tensor("wf1T_q", [C, HS], BF16, kind="ExternalInput")
    wf2T_q = nc.dram_tensor("wf2T_q", [HS, C], BF16, kind="ExternalInput")
    # per-group dequant consts: mean_c[g]/127 for g in (qkv, proj, fc1, fc2)
    wsc127 = nc.dram_tensor("wsc127", [4], F32, kind="ExternalInput")
    bqk = nc.dram_tensor("bqk", [2 * CS], F32, kind="ExternalInput")
    bv = nc.dram_tensor("bv", [CS], F32, kind="ExternalInput")
    bp = nc.dram_tensor("bp", [C], F32, kind="ExternalInput")
    bf1 = nc.dram_tensor("bf1", [HS], F32, kind="ExternalInput")
    bf2 = nc.dram_tensor("bf2", [C], F32, kind="ExternalInput")
    g1 = be1 = g2 = be2 = None
    if not g1_trivial:
        g1 = nc.dram_tensor("g1", [C], F32, kind="ExternalInput")
        be1 = nc.dram_tensor("be1", [C], F32, kind="ExternalInput")
    if not g2_trivial:
        g2 = nc.dram_tensor("g2", [C], F32, kind="ExternalInput")
        be2 = nc.dram_tensor("be2", [C], F32, kind="ExternalInput")
    onehot = nc.dram_tensor("onehot", [TP], F32, kind="ExternalInput")
    rank_in = nc.dram_tensor("rank_in", [1, 1], mybir.dt.uint32,
                             kind="ExternalInput")
    y_sh = nc.dram_tensor("y_sh", [TOK, C], F32, kind="ExternalOutput")

    with tile.TileContext(nc) as tc:
        import contextlib
        with contextlib.ExitStack() as ctx:
            dram = ctx.enter_context(tc.tile_pool(name="dram", bufs=1, space="DRAM"))
            consts = ctx.enter_context(tc.tile_pool(name="consts", bufs=1))
            wres = ctx.enter_context(tc.tile_pool(name="wres", bufs=1))
            acts = ctx.enter_context(tc.tile_pool(name="acts", bufs=1))
            big = ctx.enter_context(tc.tile_pool(name="big", bufs=1))
            rowp = ctx.enter_context(tc.tile_pool(name="rowp", bufs=1))
            t8 = ctx.enter_context(tc.tile_pool(name="t8", bufs=2))
            t4 = ctx.enter_context(tc.tile_pool(name="t4", bufs=2))
            t2 = ctx.enter_context(tc.tile_pool(name="t2", bufs=3))
            t1 = ctx.enter_context(tc.tile_pool(name="t1", bufs=6))
            brow = ctx.enter_context(tc.tile_pool(name="brow", bufs=2))
            sm = ctx.enter_context(tc.tile_pool(name="sm", bufs=2))
            psp = ctx.enter_context(tc.tile_pool(name="psp", bufs=2, space="PSUM"))
            psa = ctx.enter_context(tc.tile_pool(name="psa", bufs=1, space="PSUM"))

            # ---------- DRAM internal buffers ----------
            def dt(name, shape, dtype):
                return dram.tile(shape, dtype, name=name)

            HTOK = TOK // 2  # 256 tokens per AG half
            BLK = HTOK * C + 2 * HTOK  # payload + f32 scales as bf16 pairs
            ag1_in = [dt("ag1_in0", [BLK], BF16), dt("ag1_in1", [BLK], BF16)]
            ag1_out = [dt("ag1_out0", [TP * BLK], BF16),
                       dt("ag1_out1", [TP * BLK], BF16)]
            ag2_in = [dt("ag2_in0", [BLK], BF16), dt("ag2_in1", [BLK], BF16)]
            ag2_out = [dt("ag2_out0", [TP * BLK], BF16),
                       dt("ag2_out1", [TP * BLK], BF16)]
            l_dram = dt("l_dram", [HPC, N], F32)
            ago_in = dt("ago_in", [N], F32)
            ago_out = dt("ago_out", [TP * N], F32)
            agg_in = dt("agg_in", [N], F32)
            agg_out = dt("agg_out", [TP * N], F32)
            ago2_in = dt("ago2_in", [CS * N], BF16)
            ago2_out = dt("ago2_out", [TP * CS * N], BF16)
            rs2c_in = [dt(f"rs2c_in{c}", [TP * P, C], BF16) for c in range(4)]
            rs2c_out = [dt(f"rs2c_out{c}", [P, C], BF16) for c in range(4)]
            lrec_dram = dt("lrec_dram", [HPC, N], F32)
            sg_dram = dt("sg_dram", [N], F32)

            # ---------- constants / bias rows ----------
            ones_col = consts.tile([P, 1], F32, name="ones_col")
            nc.vector.memset(ones_col[:], 1.0)
            eps_col = consts.tile([P, 1], F32, name="eps_col")
            nc.vector.memset(eps_col[:], EPS)
            ident = consts.tile([P, P], F32, name="ident")
            make_identity(nc, ident[:])
            ones_row = consts.tile([1, P], F32, name="ones_row")
            nc.vector.memset(ones_row[:], 1.0)

            def bcast_row(dram_ap, n, name, pool=None, tag=None):
                if pool is None:
                    r = consts.tile([P, n], F32, name=name)
                else:
                    r = pool.tile([P, 1024], F32, name=name, tag=tag or "brow")[:, :n]
                nc.sync.dma_start(r[:], dram_ap[None, :].to_broadcast((P, n)))
                return r

            bv_row = bcast_row(bv[:], CS, "bv_row")
            bqk_col = consts.tile([P, 4], F32, name="bqk_col")
            nc.sync.dma_start(bqk_col[:], bqk[:].rearrange("(j p) -> p j", p=P))
            bf1_col = consts.tile([P, KT], F32, name="bf1_col")
            nc.sync.dma_start(bf1_col[:], bf1[:].rearrange("(o p) -> p o", p=P))
            oh_bc = consts.tile([P, TP], F32, name="oh_bc")
            nc.sync.dma_start(oh_bc[:], onehot[None, :].to_broadcast((P, TP)))
            wsc_bc = consts.tile([P, 4], F32, name="wsc_bc")
            nc.sync.dma_start(wsc_bc[:], wsc127[None, :].to_broadcast((P, 4)))

            def own_select(dst, col_g):
                # dst[P, TT_LOC] = rank-selected block of col_g[P, TT_ALL]
                tmp_os = sm.tile([P, TT_LOC], F32, tag="ownsel")
                for r in range(TP):
                    src = col_g[:, TT_LOC * r:TT_LOC * (r + 1)]
                    if r == 0:
                        nc.vector.tensor_scalar(dst, src, oh_bc[:, 0:1], None,
                                                op0=ALU.mult)
                    else:
                        nc.vector.tensor_scalar(tmp_os[:], src,
                                                oh_bc[:, r:r + 1], None,
                                                op0=ALU.mult)
                        nc.vector.tensor_tensor(dst, dst, tmp_os[:], ALU.add)

            # ---------- resident quantized weights (DMA only) ----------
            # rank register for dynamic own-slice DMAs
            rank_reg = nc.sync.alloc_register("rank_reg")
            nc.sync.reg_load(rank_reg, rank_in[0:1, 0:1])
            rank_sv = nc.sync.snap(rank_reg, donate=True, min_val=0,
                                   max_val=TP - 1)

            wqk_bf = wres.tile([P, KT, 2 * CS], BF16, name="wqk_bf")   # 8KB
            wv_bf = wres.tile([P, KT, CS], BF16, name="wv_bf")         # 4KB
            wf1_bf = wres.tile([P, KT, HS], BF16, name="wf1_bf")       # 16KB
            wf2_bf = wres.tile([P, HS // P, C], BF16, name="wf2_bf")   # 16KB

            # ---------- LN + act_quant (own 512 tokens) ----------
            def ln_quant(x_tile, g_row, be_row, trivial, qout_bf, m_out):
                st6 = sm.tile([P, 2, 6], F32, tag="bnst")
                nc.vector.bn_stats(st6[:, 0, :], x_tile[:, 0:C // 2])
                nc.vector.bn_stats(st6[:, 1, :], x_tile[:, C // 2:C])
                agg = sm.tile([P, 2], F32, tag="bnagg")
                nc.vector.bn_aggr(agg[:], st6[:])
                rstd = sm.tile([P, 1], F32, tag="rstd")
                nc.scalar.activation(rstd[:], agg[:, 1:2], ACTF.Sqrt, bias=eps_col[:])
                nc.vector.reciprocal(rstd[:], rstd[:])
                h = t4.tile([P, C], F32, tag="t4f32")
                nc.vector.tensor_scalar(h[:], x_tile, agg[:, 0:1], rstd[:],
                                        op0=ALU.subtract, op1=ALU.mult)
                if not trivial:
                    nc.vector.tensor_tensor(h[:], h[:], g_row[:, :C], ALU.mult)
                    nc.vector.tensor_tensor(h[:], h[:], be_row[:, :C], ALU.add)
                nc.vector.tensor_reduce(m_out, h[:], axis=AX.X, op=ALU.max,
                                        apply_absolute_value=True)
                nc.vector.tensor_scalar(m_out, m_out, EPS, None, op0=ALU.max)
                s = sm.tile([P, 1], F32, tag="qs")
                nc.vector.reciprocal(s[:], m_out)
                nc.vector.tensor_scalar(s[:], s[:], 127.0, None, op0=ALU.mult)
                nc.vector.tensor_scalar(h[:], h[:], s[:], MAGIC,
                                        op0=ALU.mult, op1=ALU.add)
                nc.scalar.activation(qout_bf, h[:], ACTF.Copy, bias=-MAGIC)

            g1_row = be1_row = None
            if not g1_trivial:
                g1_row = bcast_row(g1[:], C, "g1_row", pool=brow)
                be1_row = bcast_row(be1[:], C, "be1_row", pool=brow)
            m1_loc = sm.tile([P, TT_LOC], F32, name="m1_loc")
            # prefetch x tiles on a dedicated slot set so the loads never
            # queue behind the AG-input writes or the ACT quant copies
            xts = []
            for j in range(TT_LOC):
                xt = t4.tile([P, C], F32, tag="xt", bufs=2)
                nc.scalar.dma_start(xt[:], x_sh[j * P:(j + 1) * P, :])
                xts.append(xt)
            for j in range(TT_LOC):
                q1t = t2.tile([P, C], BF16, tag="t2bf")
                ln_quant(xts[j][:], g1_row, be1_row, g1_trivial, q1t[:],
                         m1_loc[:, j:j + 1])
                nc.sync.dma_start(
                    ag1_in[j // 2][0:HTOK * C]
                    .rearrange("(j p c) -> p j c", p=P, c=C)[:, j % 2, :], q1t[:])
                nc.sync.dma_start(
                    ag1_in[j // 2][HTOK * C:BLK].bitcast(F32)
                    .rearrange("(j p) -> p j", p=P)[:, j % 2:j % 2 + 1],
                    m1_loc[:, j:j + 1])
                if j % 2 == 1:
                    nc.gpsimd.collective_compute(
                        "AllGather", ALU.bypass, replica_groups=G4,
                        ins=[ag1_in[j // 2].opt()],
                        outs=[ag1_out[j // 2].opt()])

            # weight loads on the scalar (ACT) HWDGE queue, issued after the
            # LN1 ACT ops so they don't delay the first AllGather; the sync
            # queue stays free for x/AG1, the gpsimd queue for collectives
            nc.scalar.dma_start(wqk_bf[:],
                                wqkT_q[:].rearrange("(o p) c -> p o c", p=P))
            nc.scalar.dma_start(wv_bf[:],
                                wvT_q[:].rearrange("(o p) c -> p o c", p=P))
            nc.scalar.dma_start(wf1_bf[:],
                                wf1T_q[:].rearrange("(o p) c -> p o c", p=P))
            nc.scalar.dma_start(wf2_bf[:],
                                wf2T_q[:].rearrange("(o p) c -> p o c", p=P))

            # dequant rows/cols from gathered scales (x wsc127[0]);
            # built PER AG-HALF so hf=0 compute never waits on AG1[1]
            rtmp = rowp.tile([P, N], F32, tag="rowtmp")
            m1_col = sm.tile([P, TT_ALL], F32, name="m1_col")
            rinv1_col = sm.tile([P, TT_ALL], F32, name="rinv1_col")
            rinv1_bc = rtmp
            for hf in range(2):
                for r in range(TP):
                    sc_r = ag1_out[hf][r * BLK + HTOK * C:(r + 1) * BLK].bitcast(F32)
                    toff = r * TOK + hf * HTOK
                    # tiny row load + PE ones-broadcast instead of a slow
                    # 128-way DMA broadcast on the sync queue
                    scr = rowp.tile([1, 512], F32, tag="rowper",
                                    name="scrow", bufs=3)[:, :HTOK]
                    nc.sync.dma_start(scr, sc_r[None, :])
                    bcp = psp.tile([P, HTOK], F32, tag="pb")
                    nc.tensor.matmul(bcp[:], ones_row[:], scr,
                                     start=True, stop=True)
                    nc.vector.tensor_scalar(
                        rinv1_bc[:, toff:toff + HTOK], bcp[:],
                        wsc_bc[:, 0:1], None, op0=ALU.mult)
                    joff = r * TT_LOC + hf * 2
                    nc.sync.dma_start(m1_col[:, joff:joff + 2],
                                      sc_r.rearrange("(j p) -> p j", p=P))
                    nc.vector.tensor_scalar(
                        rinv1_col[:, joff:joff + 2],
                        m1_col[:, joff:joff + 2], wsc_bc[:, 0:1],
                        None, op0=ALU.mult)

            # ---------- QKV ----------
            qk_bf = acts.tile([P, 4, N], BF16, name="qk_bf")
            v_aug = acts.tile([P, TT_ALL, HPC, DH + 1], BF16, name="v_aug")
            nc.vector.memset(v_aug[:, :, :, DH:DH + 1], 1.0)

            for hf in range(2):
                for t1c in range(4):
                    sl = slice(t1c * 512 + hf * HTOK,
                               t1c * 512 + (hf + 1) * HTOK)
                    q1T = t8.tile([P, KT, HTOK], BF16, tag="t8bf")
                    nc.sync.dma_start_transpose(
                        q1T[:],
                        ag1_out[hf][t1c * BLK:t1c * BLK + HTOK * C]
                        .rearrange("(t c) -> t c", c=C))
                    for jt in range(4):
                        pqk = psp.tile([P, HTOK], F32, tag="pb")
                        for ct in range(KT):
                            nc.tensor.matmul(pqk[:],
                                             wqk_bf[:, ct, jt * P:(jt + 1) * P],
                                             q1T[:, ct, :], start=(ct == 0),
                                             stop=(ct == KT - 1))
                        dq = t2.tile([P, HTOK], F32, tag="t2f32")
                        nc.vector.tensor_tensor(dq[:], pqk[:], rinv1_bc[:, sl],
                                                ALU.mult)
                        nc.vector.tensor_scalar(qk_bf[:, jt, sl], dq[:],
                                                bqk_col[:, jt:jt + 1], None,
                                                op0=ALU.add)
                    for k2 in range(2):
                        tt = t1c * 4 + hf * 2 + k2
                        pv = psp.tile([P, 512], F32, tag="pb")
                        for ct in range(KT):
                            nc.tensor.matmul(pv[:, 0:CS],
                                             q1T[:, ct, k2 * P:(k2 + 1) * P],
                                             wv_bf[:, ct, :], start=(ct == 0),
                                             stop=(ct == KT - 1))
                        vdq = t1.tile([P, CS], F32, tag="t1f32")
                        nc.vector.tensor_scalar(vdq[:], pv[:, 0:CS],
                                                rinv1_col[:, tt:tt + 1], None,
                                                op0=ALU.mult)
                        nc.vector.tensor_tensor(
                            v_aug[:, tt, :, 0:DH],
                            vdq[:].rearrange("p (h d) -> p h d", d=DH),
                            bv_row[:].rearrange("p (h d) -> p h d", d=DH),
                            ALU.add)

            # ---------- attention ----------
            o_un = big.tile([P, HPC // 2, N], F32, tag="bigf32")
            moc = sm.tile([P, TT_ALL, HPC], F32, name="moc")
            lcol = sm.tile([P, TT_ALL, HPC], F32, name="lcol")
            SCALE = DH ** -0.5
            for hp in range(HPC // 2):
                h_e, h_o = 2 * hp, 2 * hp + 1
                for t1c in range(4):
                    sl = slice(t1c * 512, (t1c + 1) * 512)
                    po_e = psa.tile([P, 512], F32, tag="po_e")
                    po_o = psa.tile([P, 512], F32, tag="po_o")
                    for tt2 in range(TT_ALL):
                        sreg = psp.tile([P, 2, 512], F32, tag="sreg", bufs=2)
                        for ii, hh in enumerate((h_e, h_o)):
                            jk = CS + DH * hh
                            jq = DH * hh
                            kT_ap = qk_bf[(jk % P):(jk % P) + DH, jk // P,
                                          tt2 * P:(tt2 + 1) * P]
                            qT_ap = qk_bf[(jq % P):(jq % P) + DH, jq // P, sl]
                            nc.tensor.matmul(sreg[:, ii, :], kT_ap, qT_ap,
                                             start=True, stop=True)
                        pt = t1.tile([P, 2, 512], BF16, tag="ptbf", bufs=4)
                        nc.scalar.activation(pt[:], sreg[:], ACTF.Exp, scale=SCALE)
                        nc.tensor.matmul(po_e[0:DH + 1, :], v_aug[:, tt2, h_e, :],
                                         pt[:, 0, :], start=(tt2 == 0),
                                         stop=(tt2 == TT_ALL - 1),
                                         skip_group_check=True)
                        nc.tensor.matmul(po_o[0:DH + 1, :], v_aug[:, tt2, h_o, :],
                                         pt[:, 1, :], start=(tt2 == 0),
                                         stop=(tt2 == TT_ALL - 1),
                                         skip_group_check=True)
                    nc.vector.tensor_copy(o_un[0:DH, hp, sl], po_e[0:DH, :])
                    nc.vector.tensor_copy(o_un[DH:2 * DH, hp, sl], po_o[0:DH, :])
                    lr = t2.tile([P, 512], F32, tag="t2f32")
                    nc.vector.tensor_copy(lr[DH:DH + 1, :], po_e[DH:DH + 1, :])
                    lr2 = t2.tile([P, 512], F32, tag="t2f32")
                    nc.vector.tensor_copy(lr2[DH:DH + 1, :], po_o[DH:DH + 1, :])
                    nc.sync.dma_start(l_dram[h_e, sl], lr[DH:DH + 1, :])
                    nc.sync.dma_start(l_dram[h_o, sl], lr2[DH:DH + 1, :])
                # per-pair absmax stats as soon as the pair finishes
                for tb in range(TT_ALL):
                    tr_ps = psp.tile([P, 512], F32, tag="pb")
                    nc.tensor.transpose(tr_ps[:, 0:P],
                                        o_un[:, hp, tb * P:(tb + 1) * P],
                                        ident[:])
                    nc.vector.tensor_reduce(
                        moc[:, tb, 2 * hp:2 * hp + 2],
                        tr_ps[:, 0:P].rearrange("p (h d) -> p h d", d=DH),
                        axis=AX.X, op=ALU.max, apply_absolute_value=True)
                for hh in (h_e, h_o):
                    nc.sync.dma_start(lcol[:, :, hh],
                                      l_dram[hh, :]
                                      .rearrange("(j p) -> p j", p=P))
                nc.vector.reciprocal(lcol[:, :, h_e:h_o + 1],
                                     lcol[:, :, h_e:h_o + 1])
                nc.vector.tensor_tensor(moc[:, :, h_e:h_o + 1],
                                        moc[:, :, h_e:h_o + 1],
                                        lcol[:, :, h_e:h_o + 1], ALU.mult)

            # ---------- o absmax + quant ----------
            mo_col = sm.tile([P, TT_ALL], F32, name="mo_col")
            nc.vector.tensor_reduce(mo_col[:], moc[:], axis=AX.X, op=ALU.max)
            nc.vector.tensor_scalar(mo_col[:], mo_col[:], EPS, None, op0=ALU.max)
            nc.sync.dma_start(ago_in[:].rearrange("(j p) -> p j", p=P), mo_col[:])
            nc.gpsimd.collective_compute(
                "AllGather", ALU.bypass, replica_groups=G4,
                ins=[ago_in.opt()], outs=[ago_out.opt()])
            mo_all = sm.tile([P, TT_ALL, TP], F32, name="mo_all")
            for r in range(TP):
                nc.sync.dma_start(
                    mo_all[:, :, r],
                    ago_out[r * N:(r + 1) * N].rearrange("(j p) -> p j", p=P))
            mo_colg = sm.tile([P, TT_ALL], F32, name="mo_colg")
            nc.vector.tensor_reduce(mo_colg[:], mo_all[:], axis=AX.X, op=ALU.max)

            so_col = sm.tile([P, TT_ALL], F32, name="so_col")
            nc.vector.reciprocal(so_col[:], mo_colg[:])
            nc.vector.tensor_scalar(so_col[:], so_col[:], 127.0, None,
                                    op0=ALU.mult)
            # rowf[t, h] = so[t] * (1/l_h[t])  (col space), to DRAM rows
            rowf_col = sm.tile([P, TT_ALL, HPC], F32, name="rowf_col")
            nc.vector.tensor_tensor(rowf_col[:], lcol[:],
                                    so_col[:, :, None].to_broadcast(
                                        (P, TT_ALL, HPC)), ALU.mult)
            for hh in range(HPC):
                nc.sync.dma_start(lrec_dram[hh, :].rearrange("(j p) -> p j", p=P),
                                  rowf_col[:, :, hh])

            oq = acts.tile([P, HPC // 2, N], BF16, name="oq")
            for hh in range(HPC):
                base = DH * (hh % 2)
                for ch in range(4):
                    csl = slice(ch * 512, (ch + 1) * 512)
                    rfr = rowp.tile([1, 512], F32, tag="rowper", name="rfr",
                                    bufs=3)
                    nc.sync.dma_start(rfr[:], lrec_dram[hh, csl][None, :])
                    bc_ps = psp.tile([P, 512], F32, tag="pb")
                    nc.tensor.matmul(bc_ps[:], ones_row[:], rfr[:],
                                     start=True, stop=True)
                    tq = t2.tile([P, 512], F32, tag="t2f32")
                    nc.vector.tensor_tensor(tq[base:base + DH, :],
                                            o_un[base:base + DH, hh // 2, csl],
                                            bc_ps[base:base + DH, :], ALU.mult)
                    nc.vector.tensor_scalar(tq[base:base + DH, :],
                                            tq[base:base + DH, :], MAGIC, None,
                                            op0=ALU.add)
                    nc.scalar.activation(oq[base:base + DH, hh // 2, csl],
                                         tq[base:base + DH, :], ACTF.Copy,
                                         bias=-MAGIC)

            # ---------- AllGather oq (channel shards) ----------
            # wp aliases the dead wqk slot; its load waits out attention and
            # overlaps the oq AllGather
            wp_bf = wres.tile([P, KT, C], BF16, name="wqk_bf")         # 16KB
            nc.scalar.dma_start(wp_bf[:],
                                wpT_q[:].rearrange("(o p) c -> p o c", p=P))
            nc.sync.dma_start(
                ago2_in[:].rearrange("(cc p t) -> p cc t", cc=2, p=P, t=N),
                oq[:])
            nc.gpsimd.collective_compute(
                "AllGather", ALU.bypass, replica_groups=G4,
                ins=[ago2_in.opt()], outs=[ago2_out.opt()])

            # x_mid = x + bp prep overlaps the oq AllGather (no dep on it)
            rinvo_own = sm.tile([P, TT_LOC], F32, name="rinvo_own")
            own_select(rinvo_own[:], mo_colg[:])
            nc.vector.tensor_scalar(rinvo_own[:], rinvo_own[:],
                                    wsc_bc[:, 1:2], None, op0=ALU.mult)
            x_mid = big.tile([P, TT_LOC, C], F32, tag="bigf32")
            bp_row = bcast_row(bp[:], C, "bp_row", pool=brow)
            for j in range(TT_LOC):
                xt0 = t4.tile([P, C], F32, tag="xt", bufs=2)
                nc.scalar.dma_start(xt0[:], x_sh[j * P:(j + 1) * P, :])
                nc.vector.tensor_tensor(x_mid[:, j, :], xt0[:], bp_row[:, :C],
                                        ALU.add)

            # gathered block r = rank r's 256 channels x all tokens; pick own
            # 512-token slice with a rank-register dynamic DMA offset
            oq_full = acts.tile([P, KT, TOK], BF16, name="v_aug")
            nc.sync.dma_start(
                oq_full[:],
                ago2_out[:].rearrange("(r cc p cand t) -> p (r cc) cand t",
                                      r=TP, cc=2, p=P, cand=TP, t=TOK)
                [:, :, bass.ds(rank_sv, 1), :]
                .rearrange("p rcc one t -> p rcc (one t)"))

            # ---------- proj + LN2 + quant ----------
            g2_row = be2_row = None
            if not g2_trivial:
                g2_row = bcast_row(g2[:], C, "g2_row", pool=brow)
                be2_row = bcast_row(be2[:], C, "be2_row", pool=brow)
            m2_loc = sm.tile([P, TT_LOC], F32, name="m2_loc")
            for j in range(TT_LOC):
                xm = x_mid[:, j, :]
                for half in range(2):
                    pp = psp.tile([P, 512], F32, tag="pb")
                    for ct in range(KT):
                        nc.tensor.matmul(
                            pp[:], oq_full[:, ct, j * P:(j + 1) * P],
                            wp_bf[:, ct, half * 512:(half + 1) * 512],
                            start=(ct == 0), stop=(ct == KT - 1))
                    dqt = t2.tile([P, 512], F32, tag="t2f32")
                    nc.vector.tensor_scalar(dqt[:], pp[:],
                                            rinvo_own[:, j:j + 1],
                                            None, op0=ALU.mult)
                    nc.vector.tensor_tensor(xm[:, half * 512:(half + 1) * 512],
                                            xm[:, half * 512:(half + 1) * 512],
                                            dqt[:], ALU.add)
                q2t = t2.tile([P, C], BF16, tag="t2bf")
                ln_quant(xm, g2_row, be2_row, g2_trivial, q2t[:],
                         m2_loc[:, j:j + 1])
                nc.sync.dma_start(
                    ag2_in[j // 2][0:HTOK * C]
                    .rearrange("(j p c) -> p j c", p=P, c=C)[:, j % 2, :], q2t[:])
                nc.sync.dma_start(
                    ag2_in[j // 2][HTOK * C:BLK].bitcast(F32)
                    .rearrange("(j p) -> p j", p=P)[:, j % 2:j % 2 + 1],
                    m2_loc[:, j:j + 1])
                if j % 2 == 1:
                    nc.gpsimd.collective_compute(
                        "AllGather", ALU.bypass, replica_groups=G4,
                        ins=[ag2_in[j // 2].opt()],
                        outs=[ag2_out[j // 2].opt()])

            # rinv2 as a broadcast ROW [P, N] (for hidden-major fc1 dequant)
            # and col form (for own_select at the end we need m2 too? no --
            # final dequant uses gelu-quant scale, not rinv2)
            rinv2_bc = rowp.tile([P, N], F32, tag="rowtmp")
            for hf in range(2):
                for r in range(TP):
                    sc_r = ag2_out[hf][r * BLK + HTOK * C:(r + 1) * BLK].bitcast(F32)
                    toff = r * TOK + hf * HTOK
                    scr = rowp.tile([1, 512], F32, tag="rowper",
                                    name="scrow2", bufs=3)[:, :HTOK]
                    nc.sync.dma_start(scr, sc_r[None, :])
                    bcp = psp.tile([P, HTOK], F32, tag="pb")
                    nc.tensor.matmul(bcp[:], ones_row[:], scr,
                                     start=True, stop=True)
                    nc.vector.tensor_scalar(
                        rinv2_bc[:, toff:toff + HTOK], bcp[:],
                        wsc_bc[:, 2:3], None, op0=ALU.mult)

            # ---------- fc1 (hidden-major output) + gelu, all in SBUF ----
            # gelA aliases qk_bf (dead after attention): same pool/tag/shape
            gelA = acts.tile([P, 4, N], BF16, name="qk_bf")
            gelB = acts.tile([P, 4, N], BF16, name="gelB")

            def gel(ht):
                return (gelA if ht < 4 else gelB)[:, ht % 4, :]

            mg_col = sm.tile([P, TT_ALL], F32, name="mg_col")
            for hf in range(2):
                for t1c in range(4):
                    sl = slice(t1c * 512 + hf * HTOK,
                               t1c * 512 + (hf + 1) * HTOK)
                    q2T = t8.tile([P, KT, HTOK], BF16, tag="t8bf")
                    nc.sync.dma_start_transpose(
                        q2T[:],
                        ag2_out[hf][t1c * BLK:t1c * BLK + HTOK * C]
                        .rearrange("(t c) -> t c", c=C))
                    for ht in range(KT):
                        phT = psp.tile([P, HTOK], F32, tag="pb")
                        for ct in range(KT):
                            nc.tensor.matmul(
                                phT[:], wf1_bf[:, ct, ht * P:(ht + 1) * P],
                                q2T[:, ct, :], start=(ct == 0),
                                stop=(ct == KT - 1))
                        gt = t2.tile([P, HTOK], F32, tag="t2f32")
                        nc.vector.tensor_tensor(gt[:], phT[:], rinv2_bc[:, sl],
                                                ALU.mult)
                        nc.scalar.activation(gel(ht)[:, sl], gt[:], ACTF.Gelu,
                                             bias=bf1_col[:, ht:ht + 1])
                    # per-token max via TT tree + PE transpose (gelu >= -.17)
                    mt = t2.tile([P, HTOK], F32, tag="t2f32")
                    nc.vector.tensor_tensor(mt[:], gel(0)[:, sl],
                                            gel(1)[:, sl], ALU.max)
                    for ht in range(2, KT):
                        nc.vector.tensor_tensor(mt[:], mt[:], gel(ht)[:, sl],
                                                ALU.max)
                    for tb in range(2):
                        tt = t1c * 4 + hf * 2 + tb
                        trm = psp.tile([P, 512], F32, tag="pb")
                        nc.tensor.transpose(trm[:, 0:P],
                                            mt[:, tb * P:(tb + 1) * P],
                                            ident[:])
                        nc.vector.tensor_reduce(mg_col[:, tt:tt + 1],
                                                trm[:, 0:P],
                                                axis=AX.X, op=ALU.max)
            nc.vector.tensor_scalar(mg_col[:], mg_col[:], GELU_MIN, None,
                                    op0=ALU.max)
            nc.sync.dma_start(agg_in[:].rearrange("(j p) -> p j", p=P), mg_col[:])
            nc.gpsimd.collective_compute(
                "AllGather", ALU.bypass, replica_groups=G4,
                ins=[agg_in.opt()], outs=[agg_out.opt()])
            mg_all = sm.tile([P, TT_ALL, TP], F32, name="mg_all")
            for r in range(TP):
                nc.sync.dma_start(
                    mg_all[:, :, r],
                    agg_out[r * N:(r + 1) * N].rearrange("(j p) -> p j", p=P))
            mg_colg = sm.tile([P, TT_ALL], F32, name="mg_colg")
            nc.vector.tensor_reduce(mg_colg[:], mg_all[:], axis=AX.X, op=ALU.max)

            # sg row: 127/mg_colg, via DRAM natural-order roundtrip
            sg_col = sm.tile([P, TT_ALL], F32, name="sg_col")
            nc.vector.reciprocal(sg_col[:], mg_colg[:])
            nc.vector.tensor_scalar(sg_col[:], sg_col[:], 127.0, None,
                                    op0=ALU.mult)
            nc.sync.dma_start(sg_dram[:].rearrange("(j p) -> p j", p=P),
                              sg_col[:])
            sg_bc = rowp.tile([P, N], F32, tag="rowtmp")
            for ch in range(4):
                csl = slice(ch * 512, (ch + 1) * 512)
                sgr = rowp.tile([1, 512], F32, tag="rowper", name="sgrow",
                                bufs=3)
                nc.sync.dma_start(sgr[:], sg_dram[csl][None, :])
                sgp = psp.tile([P, 512], F32, tag="pb")
                nc.tensor.matmul(sgp[:], ones_row[:], sgr[:],
                                 start=True, stop=True)
                nc.vector.tensor_copy(sg_bc[:, csl], sgp[:])

            # quantize gelu in place (hidden-major)
            for t1c in range(4):
                sl = slice(t1c * 512, (t1c + 1) * 512)
                for ht in range(KT):
                    gq32 = t2.tile([P, 512], F32, tag="t2f32")
                    nc.vector.tensor_tensor(gq32[:], gel(ht)[:, sl],
                                            sg_bc[:, sl], ALU.mult)
                    nc.vector.tensor_scalar(gq32[:], gq32[:], MAGIC, None,
                                            op0=ALU.add)
                    nc.scalar.activation(gel(ht)[:, sl], gq32[:], ACTF.Copy,
                                         bias=-MAGIC)

            # ---------- fc2 (raw int partials, 4-way chunked RS) ----------
            # chunk c: token tiles {4r+c}; rank r's share lands at rows r*128
            for c in range(4):
                for r in range(TP):
                    tt = 4 * r + c
                    for half in range(2):
                        pf = psp.tile([P, 512], F32, tag="pb")
                        for ct in range(HS // P):
                            nc.tensor.matmul(
                                pf[:], gel(ct)[:, tt * P:(tt + 1) * P],
                                wf2_bf[:, ct, half * 512:(half + 1) * 512],
                                start=(ct == 0), stop=(ct == HS // P - 1))
                        fcp = t1.tile([P, 512], BF16, tag="t1bf")
                        nc.vector.tensor_copy(fcp[:], pf[:])
                        nc.gpsimd.dma_start(
                            rs2c_in[c][r * P:(r + 1) * P,
                                       half * 512:(half + 1) * 512], fcp[:])
                nc.gpsimd.collective_compute(
                    "ReduceScatter", ALU.add, replica_groups=G4,
                    ins=[rs2c_in[c].opt()], outs=[rs2c_out[c].opt()])

            # ---------- final: y = x_mid + deq(rs2) + bf2 ----------
            bf2_row = bcast_row(bf2[:], C, "bf2_row", pool=brow)
            rinvg_own = sm.tile([P, TT_LOC], F32, name="rinvg_own")
            own_select(rinvg_own[:], mg_colg[:])
            nc.vector.tensor_scalar(rinvg_own[:], rinvg_own[:],
                                    wsc_bc[:, 3:4], None, op0=ALU.mult)
            for j in range(TT_LOC):
                rst = t2.tile([P, C], BF16, tag="t2bf")
                nc.sync.dma_start(rst[:], rs2c_out[j][:, :])
                yt = t4.tile([P, C], F32, tag="t4f32")
                nc.vector.tensor_scalar(yt[:], rst[:], rinvg_own[:, j:j + 1],
                                        None, op0=ALU.mult)
                nc.vector.tensor_tensor(yt[:], yt[:], bf2_row[:, :C], ALU.add)
                nc.vector.tensor_tensor(yt[:], yt[:], x_mid[:, j, :], ALU.add)
                nc.sync.dma_start(y_sh[j * P:(j + 1) * P, :], yt[:])

            # optional debug taps: copy internal DRAM buffers to outputs
            dbg_srcs = {
                "l_dram": l_dram,
                "ago_out": ago_out,
                "agg_out": agg_out,
            }
            for dname in debug_outs:
                src = dbg_srcs[dname]
                dt_out = nc.dram_tensor("dbg_" + dname, list(src.shape),
                                        src.dtype, kind="ExternalOutput")
                nc.sync.dma_start(dt_out[:], src[:])

    nc.compile()
    return nc


_CACHE = {}


def _ternarize(w):
    # per-tensor ternary absmean quantization (matches reference.weight_quant)
    mean_c = max(float(np.mean(np.abs(w), dtype=np.float64)), EPS)
    q = np.clip(np.round(w * np.float32(1.0 / mean_c)), -1, 1)
    return q.astype(np.float32), mean_c


def kernel(**inputs):
    m = _imports()
    import ml_dtypes
    bf16 = ml_dtypes.bfloat16

    x = np.ascontiguousarray(np.asarray(inputs["x"]), dtype=np.float32)
    assert int(inputs["num_heads"]) == H
    w_qkv = np.asarray(inputs["w_qkv"], np.float32)
    b_qkv = np.asarray(inputs["b_qkv"], np.float32)
    w_proj = np.asarray(inputs["w_proj"], np.float32)
    b_proj = np.asarray(inputs["b_proj"], np.float32)
    w_fc1 = np.asarray(inputs["w_fc1"], np.float32)
    b_fc1 = np.asarray(inputs["b_fc1"], np.float32)
    w_fc2 = np.asarray(inputs["w_fc2"], np.float32)
    b_fc2 = np.asarray(inputs["b_fc2"], np.float32)
    g1 = np.asarray(inputs["g1"], np.float32)
    be1 = np.asarray(inputs["be1"], np.float32)
    g2 = np.asarray(inputs["g2"], np.float32)
    be2 = np.asarray(inputs["be2"], np.float32)

    g1_trivial = bool(np.all(g1 == 1.0) and np.all(be1 == 0.0))
    g2_trivial = bool(np.all(g2 == 1.0) and np.all(be2 == 0.0))

    key = (g1_trivial, g2_trivial)
    if key not in _CACHE:
        _CACHE[key] = build_kernel(g1_trivial, g2_trivial)
    nc = _CACHE[key]

    qkv_q, mean_qkv = _ternarize(w_qkv)
    proj_q, mean_proj = _ternarize(w_proj)
    fc1_q, mean_fc1 = _ternarize(w_fc1)
    fc2_q, mean_fc2 = _ternarize(w_fc2)
    wsc127 = np.array([mean_qkv, mean_proj, mean_fc1, mean_fc2],
                      np.float32) / np.float32(127.0)

    in_maps = []
    for c in range(NCORES):
        g, r = divmod(c, TP)
        tok = slice(TOK * r, TOK * (r + 1))
        hsl = slice(CS * r, CS * (r + 1))
        im = {
            "x_sh": np.ascontiguousarray(x[g, tok]),
            "wqkT_q": np.ascontiguousarray(np.concatenate(
                [qkv_q[hsl, :].T, qkv_q[C:][hsl, :].T], axis=1)).astype(bf16),
            "wvT_q": np.ascontiguousarray(qkv_q[2 * C:][hsl, :].T).astype(bf16),
            "wpT_q": np.ascontiguousarray(proj_q.T).astype(bf16),
            "wf1T_q": np.ascontiguousarray(
                fc1_q[HS * r:HS * (r + 1), :].T).astype(bf16),
            "wf2T_q": np.ascontiguousarray(
                fc2_q[:, HS * r:HS * (r + 1)].T).astype(bf16),
            "wsc127": wsc127,
            "bqk": np.ascontiguousarray(
                np.concatenate([b_qkv[hsl], b_qkv[C:][hsl]])),
            "bv": np.ascontiguousarray(b_qkv[2 * C:][hsl]),
            "bp": b_proj,
            "onehot": np.eye(TP, dtype=np.float32)[r],
            "rank_in": np.array([[r]], dtype=np.uint32),
            "bf1": np.ascontiguousarray(b_fc1[HS * r:HS * (r + 1)]),
            "bf2": b_fc2,
        }
        if not g1_trivial:
            im["g1"], im["be1"] = g1, be1
        if not g2_trivial:
            im["g2"], im["be2"] = g2, be2
        in_maps.append(im)

    global _last_in_maps
    _last_in_maps = in_maps
    res = m["run"](nc, in_maps, core_ids=list(range(NCORES)))
    out = np.empty((B, N, C), np.float32)
    for c in range(NCORES):
        g, r = divmod(c, TP)
        out[g, TOK * r:TOK * (r + 1)] = res.results[c]["y_sh"]
    return out


# revision 67
# speedup vs baseline: 1.0233x; 1.0233x over previous
"""BitNet transformer block on 8 Trainium2 NeuronCores (Bass/Tile).

Sharding: DP2 (batch) x TP4 (Megatron-style, sequence-parallel norms).
Cores 0-3 -> batch 0, cores 4-7 -> batch 1. Within each group of 4:
  - weights arrive pre-ternarized (host preprocessing, like the host-side
    sharding transposes): bf16 {-1,0,+1} plus one per-tensor dequant
    scale; every matmul is an exact integer matmul in bf16 with fp32
    PSUM accumulation,
  - each core owns 512 tokens for LN + act_quant (sequence parallel);
    quantized activations (small exact ints carried as bf16) are
    AllGathered,
  - attention is head-parallel (4 heads/core) in S^T layout: exp with no
    max subtraction (scores are O(1)); P^T feeds O^T = v^T @ P^T directly;
    a ones column appended to v yields the softmax denominator,
  - proj/fc2 are row-parallel: raw integer partial sums ReduceScatter in
    bf16 and are dequantized after the reduce,
  - fc1 produces hidden-major (transposed) output directly, so gelu,
    act_quant, and fc2 all run from SBUF with no DRAM spill and no
    activation transpose.
"""

import sys

for _p in ("/opt/trn_rl_repo",):
    if _p not in sys.path:
        sys.path.append(_p)

import numpy as np

_BASS = {}


def _imports():
    if _BASS:
        return _BASS
    import concourse.bass as bass
    import concourse.bass_isa as bass_isa
    import concourse.mybir as mybir
    import concourse.tile as tile
    from concourse import bacc
    from concourse.bass_utils import run_bass_kernel_spmd
    from concourse.masks import make_identity
    _BASS.update(bass=bass, bass_isa=bass_isa, mybir=mybir, tile=tile,
                 bacc=bacc, run=run_bass_kernel_spmd, mkid=make_identity)
    return _BASS

# ---- problem constants (hardcoded per spec) ----
B, N, C, H = 2, 2048, 1024, 16
HID = 4 * C
NCORES, TP = 8, 4
TOK = N // TP            # 512 tokens per core
TT_LOC = TOK // 128      # 4
TT_ALL = N // 128        # 16
HPC = H // TP            # 4 heads per core
DH = C // H              # 64
CS = C // TP             # 256 channel shard (proj contraction)
HS = HID // TP           # 1024 hidden shard
P = 128
KT = C // P              # 8
EPS = 1e-5
MAGIC = 12582912.0       # 1.5 * 2**23: fp32 round-half-even trick
GELU_MIN = 0.17000043    # |min gelu| bound; per-token absmax = max(max, this)
G4 = [[0, 1, 2, 3], [4, 5, 6, 7]]


def build_kernel(g1_trivial, g2_trivial, debug_outs=()):
    m = _imports()
    bass, bass_isa, mybir, tile, bacc = (m["bass"], m["bass_isa"], m["mybir"],
                                         m["tile"], m["bacc"])
    F32, BF16 = mybir.dt.float32, mybir.dt.bfloat16
    AX, ALU, ACTF = (mybir.AxisListType, mybir.AluOpType,
                     mybir.ActivationFunctionType)

    make_identity = m["mkid"]
    nc = bacc.Bacc("TRN2", target_bir_lowering=False, debug=False,
                   num_devices=NCORES)

    x_sh = nc.dram_tensor("x_sh", [TOK, C], F32, kind="ExternalInput")
    # pre-ternarized weights (bf16 {-1,0,1}), pre-transposed for lhsT use
    wqkT_q = nc.dram_tensor("wqkT_q", [C, 2 * CS], BF16, kind="ExternalInput")
    wvT_q = nc.dram_tensor("wvT_q", [C, CS], BF16, kind="ExternalInput")
    wpT_q = nc.dram_tensor("wpT_q", [C, C], BF16, kind="ExternalInput")
    wf1T_q = nc.dram_tensor("wf1T_q", [C, HS], BF16, kind="ExternalInput")
    wf2T_q = nc.dram_tensor("wf2T_q", [HS, C], BF16, kind="ExternalInput")
    # per-group dequant consts: mean_c[g]/127 for g in (qkv, proj, fc1, fc2)
    wsc127 = nc.dram_tensor("wsc127", [4], F32, kind="ExternalInput")
    bqk = nc.dram_tensor("bqk", [2 * CS], F32, kind="ExternalInput")
    bv = nc.dram_tensor("bv", [CS], F32, kind="ExternalInput")
    bp = nc.dram_tensor("bp", [C], F32, kind="ExternalInput")
    bf1 = nc.dram_tensor("bf1", [HS], F32, kind="ExternalInput")
    bf2 = nc.dram_tensor("bf2", [C], F32, kind="ExternalInput")
    g1 = be1 = g2 = be2 = None
    if not g1_trivial:
        g1 = nc.dram_tensor("g1", [C], F32, kind="ExternalInput")
        be1 = nc.dram_tensor("be1", [C], F32, kind="ExternalInput")
    if not g2_trivial:
        g2 = nc.dram_tensor("g2", [C], F32, kind="ExternalInput")
        be2 = nc.dram_tensor("be2", [C], F32, kind="ExternalInput")
    onehot = nc.dram_tensor("onehot", [TP], F32, kind="ExternalInput")
    rank_in = nc.dram_tensor("rank_in", [1, 1], mybir.dt.uint32,
                             kind="ExternalInput")
    y_sh = nc.dram_tensor("y_sh", [TOK, C], F32, kind="ExternalOutput")

    with tile.TileContext(nc) as tc:
        import contextlib
        with contextlib.ExitStack() as ctx:
            dram = ctx.enter_context(tc.tile_pool(name="dram", bufs=1, space="DRAM"))
            consts = ctx.enter_context(tc.tile_pool(name="consts", bufs=1))
            wres = ctx.enter_context(tc.tile_pool(name="wres", bufs=1))
            acts = ctx.enter_context(tc.tile_pool(name="acts", bufs=1))
            big = ctx.enter_context(tc.tile_pool(name="big", bufs=1))
            rowp = ctx.enter_context(tc.tile_pool(name="rowp", bufs=1))
            t8 = ctx.enter_context(tc.tile_pool(name="t8", bufs=2))
            t4 = ctx.enter_context(tc.tile_pool(name="t4", bufs=2))
            t2 = ctx.enter_context(tc.tile_pool(name="t2", bufs=3))
            t1 = ctx.enter_context(tc.tile_pool(name="t1", bufs=6))
            brow = ctx.enter_context(tc.tile_pool(name="brow", bufs=2))
            sm = ctx.enter_context(tc.tile_pool(name="sm", bufs=2))
            psp = ctx.enter_context(tc.tile_pool(name="psp", bufs=2, space="PSUM"))
            psa = ctx.enter_context(tc.tile_pool(name="psa", bufs=1, space="PSUM"))

            # ---------- DRAM internal buffers ----------
            def dt(name, shape, dtype):
                return dram.tile(shape, dtype, name=name)

            HTOK = TOK // 2  # 256 tokens per AG half
            BLK = HTOK * C + 2 * HTOK  # payload + f32 scales as bf16 pairs
            ag1_in = [dt("ag1_in0", [BLK], BF16), dt("ag1_in1", [BLK], BF16)]
            ag1_out = [dt("ag1_out0", [TP * BLK], BF16),
                       dt("ag1_out1", [TP * BLK], BF16)]
            ag2_in = [dt("ag2_in0", [BLK], BF16), dt("ag2_in1", [BLK], BF16)]
            ag2_out = [dt("ag2_out0", [TP * BLK], BF16),
                       dt("ag2_out1", [TP * BLK], BF16)]
            l_dram = dt("l_dram", [HPC, N], F32)
            ago_in = dt("ago_in", [N], F32)
            ago_out = dt("ago_out", [TP * N], F32)
            agg_in = dt("agg_in", [N], F32)
            agg_out = dt("agg_out", [TP * N], F32)
            ago2_in = dt("ago2_in", [CS * N], BF16)
            ago2_out = dt("ago2_out", [TP * CS * N], BF16)
            rs2c_in = [dt(f"rs2c_in{c}", [TP * P, C], BF16) for c in range(4)]
            rs2c_out = [dt(f"rs2c_out{c}", [P, C], BF16) for c in range(4)]
            lrec_dram = dt("lrec_dram", [HPC, N], F32)
            sg_dram = dt("sg_dram", [N], F32)

            # ---------- constants / bias rows ----------
            ones_col = consts.tile([P, 1], F32, name="ones_col")
            nc.vector.memset(ones_col[:], 1.0)
            eps_col = consts.tile([P, 1], F32, name="eps_col")
            nc.vector.memset(eps_col[:], EPS)
            ident = consts.tile([P, P], F32, name="ident")
            make_identity(nc, ident[:])
            ones_row = consts.tile([1, P], F32, name="ones_row")
            nc.vector.memset(ones_row[:], 1.0)

            def bcast_row(dram_ap, n, name, pool=None, tag=None):
                if pool is None:
                    r = consts.tile([P, n], F32, name=name)
                else:
                    r = pool.tile([P, 1024], F32, name=name, tag=tag or "brow")[:, :n]
                nc.sync.dma_start(r[:], dram_ap[None, :].to_broadcast((P, n)))
                return r

            bv_row = bcast_row(bv[:], CS, "bv_row")
            bqk_col = consts.tile([P, 4], F32, name="bqk_col")
            nc.sync.dma_start(bqk_col[:], bqk[:].rearrange("(j p) -> p j", p=P))
            bf1_col = consts.tile([P, KT], F32, name="bf1_col")
            nc.sync.dma_start(bf1_col[:], bf1[:].rearrange("(o p) -> p o", p=P))
            oh_bc = consts.tile([P, TP], F32, name="oh_bc")
            nc.sync.dma_start(oh_bc[:], onehot[None, :].to_broadcast((P, TP)))
            wsc_bc = consts.tile([P, 4], F32, name="wsc_bc")
            nc.sync.dma_start(wsc_bc[:], wsc127[None, :].to_broadcast((P, 4)))

            def own_select(dst, col_g):
                # dst[P, TT_LOC] = rank-selected block of col_g[P, TT_ALL]
                tmp_os = sm.tile([P, TT_LOC], F32, tag="ownsel")
                for r in range(TP):
                    src = col_g[:, TT_LOC * r:TT_LOC * (r + 1)]
                    if r == 0:
                        nc.vector.tensor_scalar(dst, src, oh_bc[:, 0:1], None,
                                                op0=ALU.mult)
                    else:
                        nc.vector.tensor_scalar(tmp_os[:], src,
                                                oh_bc[:, r:r + 1], None,
                                                op0=ALU.mult)
                        nc.vector.tensor_tensor(dst, dst, tmp_os[:], ALU.add)

            # ---------- resident quantized weights (DMA only) ----------
            # rank register for dynamic own-slice DMAs
            rank_reg = nc.sync.alloc_register("rank_reg")
            nc.sync.reg_load(rank_reg, rank_in[0:1, 0:1])
            rank_sv = nc.sync.snap(rank_reg, donate=True, min_val=0,
                                   max_val=TP - 1)

            wqk_bf = wres.tile([P, KT, 2 * CS], BF16, name="wqk_bf")   # 8KB
            wv_bf = wres.tile([P, KT, CS], BF16, name="wv_bf")         # 4KB
            wf1_bf = wres.tile([P, KT, HS], BF16, name="wf1_bf")       # 16KB
            wf2_bf = wres.tile([P, HS // P, C], BF16, name="wf2_bf")   # 16KB

            # ---------- LN + act_quant (own 512 tokens) ----------
            def ln_quant(x_tile, g_row, be_row, trivial, qout_bf, m_out):
                st6 = sm.tile([P, 2, 6], F32, tag="bnst")
                nc.vector.bn_stats(st6[:, 0, :], x_tile[:, 0:C // 2])
                nc.vector.bn_stats(st6[:, 1, :], x_tile[:, C // 2:C])
                agg = sm.tile([P, 2], F32, tag="bnagg")
                nc.vector.bn_aggr(agg[:], st6[:])
                rstd = sm.tile([P, 1], F32, tag="rstd")
                nc.scalar.activation(rstd[:], agg[:, 1:2], ACTF.Sqrt, bias=eps_col[:])
                nc.vector.reciprocal(rstd[:], rstd[:])
                h = t4.tile([P, C], F32, tag="t4f32")
                nc.vector.tensor_scalar(h[:], x_tile, agg[:, 0:1], rstd[:],
                                        op0=ALU.subtract, op1=ALU.mult)
                if not trivial:
                    nc.vector.tensor_tensor(h[:], h[:], g_row[:, :C], ALU.mult)
                    nc.vector.tensor_tensor(h[:], h[:], be_row[:, :C], ALU.add)
                nc.vector.tensor_reduce(m_out, h[:], axis=AX.X, op=ALU.max,
                                        apply_absolute_value=True)
                nc.vector.tensor_scalar(m_out, m_out, EPS, None, op0=ALU.max)
                s = sm.tile([P, 1], F32, tag="qs")
                nc.vector.reciprocal(s[:], m_out)
                nc.vector.tensor_scalar(s[:], s[:], 127.0, None, op0=ALU.mult)
                nc.vector.tensor_scalar(h[:], h[:], s[:], MAGIC,
                                        op0=ALU.mult, op1=ALU.add)
                nc.scalar.activation(qout_bf, h[:], ACTF.Copy, bias=-MAGIC)

            g1_row = be1_row = None
            if not g1_trivial:
                g1_row = bcast_row(g1[:], C, "g1_row", pool=brow)
                be1_row = bcast_row(be1[:], C, "be1_row", pool=brow)
            m1_loc = sm.tile([P, TT_LOC], F32, name="m1_loc")
            # prefetch x tiles on a dedicated slot set so the loads never
            # queue behind the AG-input writes or the ACT quant copies
            xts = []
            for j in range(TT_LOC):
                xt = t4.tile([P, C], F32, tag="xt", bufs=2)
                nc.scalar.dma_start(xt[:], x_sh[j * P:(j + 1) * P, :])
                xts.append(xt)
            for j in range(TT_LOC):
                q1t = t2.tile([P, C], BF16, tag="t2bf")
                ln_quant(xts[j][:], g1_row, be1_row, g1_trivial, q1t[:],
                         m1_loc[:, j:j + 1])
                nc.sync.dma_start(
                    ag1_in[j // 2][0:HTOK * C]
                    .rearrange("(j p c) -> p j c", p=P, c=C)[:, j % 2, :], q1t[:])
                nc.sync.dma_start(
                    ag1_in[j // 2][HTOK * C:BLK].bitcast(F32)
                    .rearrange("(j p) -> p j", p=P)[:, j % 2:j % 2 + 1],
                    m1_loc[:, j:j + 1])
                if j % 2 == 1:
                    nc.gpsimd.collective_compute(
                        "AllGather", ALU.bypass, replica_groups=G4,
                        ins=[ag1_in[j // 2].opt()],
                        outs=[ag1_out[j // 2].opt()])

            # weight loads on the scalar (ACT) HWDGE queue, issued after the
            # LN1 ACT ops so they don't delay the first AllGather; the sync
            # queue stays free for x/AG1, the gpsimd queue for collectives
            nc.scalar.dma_start(wqk_bf[:],
                                wqkT_q[:].rearrange("(o p) c -> p o c", p=P))
            nc.scalar.dma_start(wv_bf[:],
                                wvT_q[:].rearrange("(o p) c -> p o c", p=P))
            nc.scalar.dma_start(wf1_bf[:],
                                wf1T_q[:].rearrange("(o p) c -> p o c", p=P))
            nc.scalar.dma_start(wf2_bf[:],
                                wf2T_q[:].rearrange("(o p) c -> p o c", p=P))

            # dequant rows/cols from gathered scales (x wsc127[0]);
            # built PER AG-HALF so hf=0 compute never waits on AG1[1]
            rtmp = rowp.tile([P, N], F32, tag="rowtmp")
            m1_col = sm.tile([P, TT_ALL], F32, name="m1_col")
            rinv1_col = sm.tile([P, TT_ALL], F32, name="rinv1_col")
            rinv1_bc = rtmp
            for hf in range(2):
                for r in range(TP):
                    sc_r = ag1_out[hf][r * BLK + HTOK * C:(r + 1) * BLK].bitcast(F32)
                    toff = r * TOK + hf * HTOK
                    # tiny row load + PE ones-broadcast instead of a slow
                    # 128-way DMA broadcast on the sync queue
                    scr = rowp.tile([1, 512], F32, tag="rowper",
                                    name="scrow", bufs=3)[:, :HTOK]
                    nc.sync.dma_start(scr, sc_r[None, :])
                    bcp = psp.tile([P, HTOK], F32, tag="pb")
                    nc.tensor.matmul(bcp[:], ones_row[:], scr,
                                     start=True, stop=True)
                    nc.vector.tensor_scalar(
                        rinv1_bc[:, toff:toff + HTOK], bcp[:],
                        wsc_bc[:, 0:1], None, op0=ALU.mult)
                    joff = r * TT_LOC + hf * 2
                    nc.sync.dma_start(m1_col[:, joff:joff + 2],
                                      sc_r.rearrange("(j p) -> p j", p=P))
                    nc.vector.tensor_scalar(
                        rinv1_col[:, joff:joff + 2],
                        m1_col[:, joff:joff + 2], wsc_bc[:, 0:1],
                        None, op0=ALU.mult)

            # ---------- QKV ----------
            qk_bf = acts.tile([P, 4, N], BF16, name="qk_bf")
            v_aug = acts.tile([P, TT_ALL, HPC, DH + 1], BF16, name="v_aug")
            nc.vector.memset(v_aug[:, :, :, DH:DH + 1], 1.0)

            for hf in range(2):
                for t1c in range(4):
                    sl = slice(t1c * 512 + hf * HTOK,
                               t1c * 512 + (hf + 1) * HTOK)
                    q1T = t8.tile([P, KT, HTOK], BF16, tag="t8bf")
                    nc.sync.dma_start_transpose(
                        q1T[:],
                        ag1_out[hf][t1c * BLK:t1c * BLK + HTOK * C]
                        .rearrange("(t c) -> t c", c=C))
                    for jt in range(4):
                        pqk = psp.tile([P, HTOK], F32, tag="pb")
                        for ct in range(KT):
                            nc.tensor.matmul(pqk[:],
                                             wqk_bf[:, ct, jt * P:(jt + 1) * P],
                                             q1T[:, ct, :], start=(ct == 0),
                                             stop=(ct == KT - 1))
                        dq = t2.tile([P, HTOK], F32, tag="t2f32")
                        nc.vector.tensor_tensor(dq[:], pqk[:], rinv1_bc[:, sl],
                                                ALU.mult)
                        nc.vector.tensor_scalar(qk_bf[:, jt, sl], dq[:],
                                                bqk_col[:, jt:jt + 1], None,
                                                op0=ALU.add)
                    for k2 in range(2):
                        tt = t1c * 4 + hf * 2 + k2
                        pv = psp.tile([P, 512], F32, tag="pb")
                        for ct in range(KT):
                            nc.tensor.matmul(pv[:, 0:CS],
                                             q1T[:, ct, k2 * P:(k2 + 1) * P],
                                             wv_bf[:, ct, :], start=(ct == 0),
                                             stop=(ct == KT - 1))
                        vdq = t1.tile([P, CS], F32, tag="t1f32")
                        nc.vector.tensor_scalar(vdq[:], pv[:, 0:CS],
                                                rinv1_col[:, tt:tt + 1], None,
                                                op0=ALU.mult)
                        nc.vector.tensor_tensor(
                            v_aug[:, tt, :, 0:DH],
                            vdq[:].rearrange("p (h d) -> p h d", d=DH),
                            bv_row[:].rearrange("p (h d) -> p h d", d=DH),
                            ALU.add)

            # ---------- attention ----------
            o_un = big.tile([P, HPC // 2, N], F32, tag="bigf32")
            moc = sm.tile([P, TT_ALL, HPC], F32, name="moc")
            lcol = sm.tile([P, TT_ALL, HPC], F32, name="lcol")
            SCALE = DH ** -0.5
            for hp in range(HPC // 2):
                h_e, h_o = 2 * hp, 2 * hp + 1
                for t1c in range(4):
                    sl = slice(t1c * 512, (t1c + 1) * 512)
                    po_e = psa.tile([P, 512], F32, tag="po_e")
                    po_o = psa.tile([P, 512], F32, tag="po_o")
                    for tt2 in range(TT_ALL):
                        sreg = psp.tile([P, 2, 512], F32, tag="sreg", bufs=2)
                        for ii, hh in enumerate((h_e, h_o)):
                            jk = CS + DH * hh
                            jq = DH * hh
                            kT_ap = qk_bf[(jk % P):(jk % P) + DH, jk // P,
                                          tt2 * P:(tt2 + 1) * P]
                            qT_ap = qk_bf[(jq % P):(jq % P) + DH, jq // P, sl]
                            nc.tensor.matmul(sreg[:, ii, :], kT_ap, qT_ap,
                                             start=True, stop=True)
                        pt = t1.tile([P, 2, 512], BF16, tag="ptbf", bufs=4)
                        nc.scalar.activation(pt[:], sreg[:], ACTF.Exp, scale=SCALE)
                        nc.tensor.matmul(po_e[0:DH + 1, :], v_aug[:, tt2, h_e, :],
                                         pt[:, 0, :], start=(tt2 == 0),
                                         stop=(tt2 == TT_ALL - 1),
                                         skip_group_check=True)
                        nc.tensor.matmul(po_o[0:DH + 1, :], v_aug[:, tt2, h_o, :],
                                         pt[:, 1, :], start=(tt2 == 0),
                                         stop=(tt2 == TT_ALL - 1),
                                         skip_group_check=True)
                    nc.vector.tensor_copy(o_un[0:DH, hp, sl], po_e[0:DH, :])
                    nc.vector.tensor_copy(o_un[DH:2 * DH, hp, sl], po_o[0:DH, :])
                    lr = t2.tile([P, 512], F32, tag="t2f32")
                    nc.vector.tensor_copy(lr[DH:DH + 1, :], po_e[DH:DH + 1, :])
                    lr2 = t2.tile([P, 512], F32, tag="t2f32")
                    nc.vector.tensor_copy(lr2[DH:DH + 1, :], po_o[DH:DH + 1, :])
                    nc.sync.dma_start(l_dram[h_e, sl], lr[DH:DH + 1, :])
                    nc.sync.dma_start(l_dram[h_o, sl], lr2[DH:DH + 1, :])
                # per-pair absmax stats as soon as the pair finishes
                for tb in range(TT_ALL):
                    tr_ps = psp.tile([P, 512], F32, tag="pb")
                    nc.tensor.transpose(tr_ps[:, 0:P],
                                        o_un[:, hp, tb * P:(tb + 1) * P],
                                        ident[:])
                    nc.vector.tensor_reduce(
                        moc[:, tb, 2 * hp:2 * hp + 2],
                        tr_ps[:, 0:P].rearrange("p (h d) -> p h d", d=DH),
                        axis=AX.X, op=ALU.max, apply_absolute_value=True)
                for hh in (h_e, h_o):
                    nc.sync.dma_start(lcol[:, :, hh],
                                      l_dram[hh, :]
                                      .rearrange("(j p) -> p j", p=P))
                nc.vector.reciprocal(lcol[:, :, h_e:h_o + 1],
                                     lcol[:, :, h_e:h_o + 1])
                nc.vector.tensor_tensor(moc[:, :, h_e:h_o + 1],
                                        moc[:, :, h_e:h_o + 1],
                                        lcol[:, :, h_e:h_o + 1], ALU.mult)

            # ---------- o absmax + quant ----------
            mo_col = sm.tile([P, TT_ALL], F32, name="mo_col")
            nc.vector.tensor_reduce(mo_col[:], moc[:], axis=AX.X, op=ALU.max)
            nc.vector.tensor_scalar(mo_col[:], mo_col[:], EPS, None, op0=ALU.max)
            nc.sync.dma_start(ago_in[:].rearrange("(j p) -> p j", p=P), mo_col[:])
            nc.gpsimd.collective_compute(
                "AllGather", ALU.bypass, replica_groups=G4,
                ins=[ago_in.opt()], outs=[ago_out.opt()])
            mo_all = sm.tile([P, TT_ALL, TP], F32, name="mo_all")
            for r in range(TP):
                nc.sync.dma_start(
                    mo_all[:, :, r],
                    ago_out[r * N:(r + 1) * N].rearrange("(j p) -> p j", p=P))
            mo_colg = sm.tile([P, TT_ALL], F32, name="mo_colg")
            nc.vector.tensor_reduce(mo_colg[:], mo_all[:], axis=AX.X, op=ALU.max)

            so_col = sm.tile([P, TT_ALL], F32, name="so_col")
            nc.vector.reciprocal(so_col[:], mo_colg[:])
            nc.vector.tensor_scalar(so_col[:], so_col[:], 127.0, None,
                                    op0=ALU.mult)
            # rowf[t, h] = so[t] * (1/l_h[t])  (col space), to DRAM rows
            rowf_col = sm.tile([P, TT_ALL, HPC], F32, name="rowf_col")
            nc.vector.tensor_tensor(rowf_col[:], lcol[:],
                                    so_col[:, :, None].to_broadcast(
                                        (P, TT_ALL, HPC)), ALU.mult)
            for hh in range(HPC):
                nc.sync.dma_start(lrec_dram[hh, :].rearrange("(j p) -> p j", p=P),
                                  rowf_col[:, :, hh])

            oq = acts.tile([P, HPC // 2, N], BF16, name="oq")
            for hh in range(HPC):
                base = DH * (hh % 2)
                for ch in range(4):
                    csl = slice(ch * 512, (ch + 1) * 512)
                    rfr = rowp.tile([1, 512], F32, tag="rowper", name="rfr",
                                    bufs=3)
                    nc.sync.dma_start(rfr[:], lrec_dram[hh, csl][None, :])
                    bc_ps = psp.tile([P, 512], F32, tag="pb")
                    nc.tensor.matmul(bc_ps[:], ones_row[:], rfr[:],
                                     start=True, stop=True)
                    tq = t2.tile([P, 512], F32, tag="t2f32")
                    nc.vector.tensor_tensor(tq[base:base + DH, :],
                                            o_un[base:base + DH, hh // 2, csl],
                                            bc_ps[base:base + DH, :], ALU.mult)
                    nc.vector.tensor_scalar(tq[base:base + DH, :],
                                            tq[base:base + DH, :], MAGIC, None,
                                            op0=ALU.add)
                    nc.scalar.activation(oq[base:base + DH, hh // 2, csl],
                                         tq[base:base + DH, :], ACTF.Copy,
                                         bias=-MAGIC)

            # ---------- AllGather oq (channel shards) ----------
            # wp aliases the dead wqk slot; its load waits out attention and
            # overlaps the oq AllGather
            wp_bf = wres.tile([P, KT, C], BF16, name="wqk_bf")         # 16KB
            nc.scalar.dma_start(wp_bf[:],
                                wpT_q[:].rearrange("(o p) c -> p o c", p=P))
            nc.sync.dma_start(
                ago2_in[:].rearrange("(cc p t) -> p cc t", cc=2, p=P, t=N),
                oq[:])
            nc.gpsimd.collective_compute(
                "AllGather", ALU.bypass, replica_groups=G4,
                ins=[ago2_in.opt()], outs=[ago2_out.opt()])
            # gathered block r = rank r's 256 channels x all tokens; pick own
            # 512-token slice with a rank-register dynamic DMA offset
            oq_full = acts.tile([P, KT, TOK], BF16, name="v_aug")
            nc.sync.dma_start(
                oq_full[:],
                ago2_out[:].rearrange("(r cc p cand t) -> p (r cc) cand t",
                                      r=TP, cc=2, p=P, cand=TP, t=TOK)
                [:, :, bass.ds(rank_sv, 1), :]
                .rearrange("p rcc one t -> p rcc (one t)"))

            # ---------- x_mid = x + deq(proj) + bp ; LN2 + quant ----------
            rinvo_own = sm.tile([P, TT_LOC], F32, name="rinvo_own")
            own_select(rinvo_own[:], mo_colg[:])
            nc.vector.tensor_scalar(rinvo_own[:], rinvo_own[:],
                                    wsc_bc[:, 1:2], None, op0=ALU.mult)
            x_mid = big.tile([P, TT_LOC, C], F32, tag="bigf32")
            bp_row = bcast_row(bp[:], C, "bp_row", pool=brow)
            for j in range(TT_LOC):
                xt0 = t4.tile([P, C], F32, tag="t4f32")
                nc.sync.dma_start(xt0[:], x_sh[j * P:(j + 1) * P, :])
                nc.vector.tensor_tensor(x_mid[:, j, :], xt0[:], bp_row[:, :C],
                                        ALU.add)
            g2_row = be2_row = None
            if not g2_trivial:
                g2_row = bcast_row(g2[:], C, "g2_row", pool=brow)
                be2_row = bcast_row(be2[:], C, "be2_row", pool=brow)
            m2_loc = sm.tile([P, TT_LOC], F32, name="m2_loc")
            for j in range(TT_LOC):
                xm = x_mid[:, j, :]
                for half in range(2):
                    pp = psp.tile([P, 512], F32, tag="pb")
                    for ct in range(KT):
                        nc.tensor.matmul(
                            pp[:], oq_full[:, ct, j * P:(j + 1) * P],
                            wp_bf[:, ct, half * 512:(half + 1) * 512],
                            start=(ct == 0), stop=(ct == KT - 1))
                    dqt = t2.tile([P, 512], F32, tag="t2f32")
                    nc.vector.tensor_scalar(dqt[:], pp[:],
                                            rinvo_own[:, j:j + 1],
                                            None, op0=ALU.mult)
                    nc.vector.tensor_tensor(xm[:, half * 512:(half + 1) * 512],
                                            xm[:, half * 512:(half + 1) * 512],
                                            dqt[:], ALU.add)
                q2t = t2.tile([P, C], BF16, tag="t2bf")
                ln_quant(xm, g2_row, be2_row, g2_trivial, q2t[:],
                         m2_loc[:, j:j + 1])
                nc.sync.dma_start(
                    ag2_in[j // 2][0:HTOK * C]
                    .rearrange("(j p c) -> p j c", p=P, c=C)[:, j % 2, :], q2t[:])
                nc.sync.dma_start(
                    ag2_in[j // 2][HTOK * C:BLK].bitcast(F32)
                    .rearrange("(j p) -> p j", p=P)[:, j % 2:j % 2 + 1],
                    m2_loc[:, j:j + 1])
                if j % 2 == 1:
                    nc.gpsimd.collective_compute(
                        "AllGather", ALU.bypass, replica_groups=G4,
                        ins=[ag2_in[j // 2].opt()],
                        outs=[ag2_out[j // 2].opt()])

            # rinv2 as a broadcast ROW [P, N] (for hidden-major fc1 dequant)
            # and col form (for own_select at the end we need m2 too? no --
            # final dequant uses gelu-quant scale, not rinv2)
            rinv2_bc = rowp.tile([P, N], F32, tag="rowtmp")
            for hf in range(2):
                for r in range(TP):
                    sc_r = ag2_out[hf][r * BLK + HTOK * C:(r + 1) * BLK].bitcast(F32)
                    toff = r * TOK + hf * HTOK
                    scr = rowp.tile([1, 512], F32, tag="rowper",
                                    name="scrow2", bufs=3)[:, :HTOK]
                    nc.sync.dma_start(scr, sc_r[None, :])
                    bcp = psp.tile([P, HTOK], F32, tag="pb")
                    nc.tensor.matmul(bcp[:], ones_row[:], scr,
                                     start=True, stop=True)
                    nc.vector.tensor_scalar(
                        rinv2_bc[:, toff:toff + HTOK], bcp[:],
                        wsc_bc[:, 2:3], None, op0=ALU.mult)

            # ---------- fc1 (hidden-major output) + gelu, all in SBUF ----
            # gelA aliases qk_bf (dead after attention): same pool/tag/shape
            gelA = acts.tile([P, 4, N], BF16, name="qk_bf")
            gelB = acts.tile([P, 4, N], BF16, name="gelB")

            def gel(ht):
                return (gelA if ht < 4 else gelB)[:, ht % 4, :]

            mg_col = sm.tile([P, TT_ALL], F32, name="mg_col")
            for hf in range(2):
                for t1c in range(4):
                    sl = slice(t1c * 512 + hf * HTOK,
                               t1c * 512 + (hf + 1) * HTOK)
                    q2T = t8.tile([P, KT, HTOK], BF16, tag="t8bf")
                    nc.sync.dma_start_transpose(
                        q2T[:],
                        ag2_out[hf][t1c * BLK:t1c * BLK + HTOK * C]
                        .rearrange("(t c) -> t c", c=C))
                    for ht in range(KT):
                        phT = psp.tile([P, HTOK], F32, tag="pb")
                        for ct in range(KT):
                            nc.tensor.matmul(
                                phT[:], wf1_bf[:, ct, ht * P:(ht + 1) * P],
                                q2T[:, ct, :], start=(ct == 0),
                                stop=(ct == KT - 1))
                        gt = t2.tile([P, HTOK], F32, tag="t2f32")
                        nc.vector.tensor_tensor(gt[:], phT[:], rinv2_bc[:, sl],
                                                ALU.mult)
                        nc.scalar.activation(gel(ht)[:, sl], gt[:], ACTF.Gelu,
                                             bias=bf1_col[:, ht:ht + 1])
                    # per-token max via TT tree + PE transpose (gelu >= -.17)
                    mt = t2.tile([P, HTOK], F32, tag="t2f32")
                    nc.vector.tensor_tensor(mt[:], gel(0)[:, sl],
                                            gel(1)[:, sl], ALU.max)
                    for ht in range(2, KT):
                        nc.vector.tensor_tensor(mt[:], mt[:], gel(ht)[:, sl],
                                                ALU.max)
                    for tb in range(2):
                        tt = t1c * 4 + hf * 2 + tb
                        trm = psp.tile([P, 512], F32, tag="pb")
                        nc.tensor.transpose(trm[:, 0:P],
                                            mt[:, tb * P:(tb + 1) * P],
                                            ident[:])
                        nc.vector.tensor_reduce(mg_col[:, tt:tt + 1],
                                                trm[:, 0:P],
                                                axis=AX.X, op=ALU.max)
            nc.vector.tensor_scalar(mg_col[:], mg_col[:], GELU_MIN, None,
                                    op0=ALU.max)
            nc.sync.dma_start(agg_in[:].rearrange("(j p) -> p j", p=P), mg_col[:])
            nc.gpsimd.collective_compute(
                "AllGather", ALU.bypass, replica_groups=G4,
                ins=[agg_in.opt()], outs=[agg_out.opt()])
            mg_all = sm.tile([P, TT_ALL, TP], F32, name="mg_all")
            for r in range(TP):
                nc.sync.dma_start(
                    mg_all[:, :, r],
                    agg_out[r * N:(r + 1) * N].rearrange("(j p) -> p j", p=P))
            mg_colg = sm.tile([P, TT_ALL], F32, name="mg_colg")
            nc.vector.tensor_reduce(mg_colg[:], mg_all[:], axis=AX.X, op=ALU.max)

            # sg row: 127/mg_colg, via DRAM natural-order roundtrip
            sg_col = sm.tile([P, TT_ALL], F32, name="sg_col")
            nc.vector.reciprocal(sg_col[:], mg_colg[:])
            nc.vector.tensor_scalar(sg_col[:], sg_col[:], 127.0, None,
                                    op0=ALU.mult)
            nc.sync.dma_start(sg_dram[:].rearrange("(j p) -> p j", p=P),
                              sg_col[:])
            sg_bc = rowp.tile([P, N], F32, tag="rowtmp")
            nc.sync.dma_start(sg_bc[:], sg_dram[None, :].to_broadcast((P, N)))

            # quantize gelu in place (hidden-major)
            for t1c in range(4):
                sl = slice(t1c * 512, (t1c + 1) * 512)
                for ht in range(KT):
                    gq32 = t2.tile([P, 512], F32, tag="t2f32")
                    nc.vector.tensor_tensor(gq32[:], gel(ht)[:, sl],
                                            sg_bc[:, sl], ALU.mult)
                    nc.vector.tensor_scalar(gq32[:], gq32[:], MAGIC, None,
                                            op0=ALU.add)
                    nc.scalar.activation(gel(ht)[:, sl], gq32[:], ACTF.Copy,
                                         bias=-MAGIC)

            # ---------- fc2 (raw int partials, 4-way chunked RS) ----------
            # chunk c: token tiles {4r+c}; rank r's share lands at rows r*128
            for c in range(4):
                for r in range(TP):
                    tt = 4 * r + c
                    for half in range(2):
                        pf = psp.tile([P, 512], F32, tag="pb")
                        for ct in range(HS // P):
                            nc.tensor.matmul(
                                pf[:], gel(ct)[:, tt * P:(tt + 1) * P],
                                wf2_bf[:, ct, half * 512:(half + 1) * 512],
                                start=(ct == 0), stop=(ct == HS // P - 1))
                        fcp = t1.tile([P, 512], BF16, tag="t1bf")
                        nc.vector.tensor_copy(fcp[:], pf[:])
                        # sync queue, NOT gpsimd: the RS completion-waits
                        # block the gpsimd FIFO, which would stall chunk
                        # c+1's input writes and serialize the RS chain
                        nc.sync.dma_start(
                            rs2c_in[c][r * P:(r + 1) * P,
                                       half * 512:(half + 1) * 512], fcp[:])
                nc.gpsimd.collective_compute(
                    "ReduceScatter", ALU.add, replica_groups=G4,
                    ins=[rs2c_in[c].opt()], outs=[rs2c_out[c].opt()])

            # ---------- final: y = x_mid + deq(rs2) + bf2 ----------
            bf2_row = bcast_row(bf2[:], C, "bf2_row", pool=brow)
            rinvg_own = sm.tile([P, TT_LOC], F32, name="rinvg_own")
            own_select(rinvg_own[:], mg_colg[:])
            nc.vector.tensor_scalar(rinvg_own[:], rinvg_own[:],
                                    wsc_bc[:, 3:4], None, op0=ALU.mult)
            for j in range(TT_LOC):
                rst = t2.tile([P, C], BF16, tag="t2bf")
                nc.sync.dma_start(rst[:], rs2c_out[j][:, :])
                yt = t4.tile([P, C], F32, tag="t4f32")
                nc.vector.tensor_scalar(yt[:], rst[:], rinvg_own[:, j:j + 1],
                                        None, op0=ALU.mult)
                nc.vector.tensor_tensor(yt[:], yt[:], bf2_row[:, :C], ALU.add)
                nc.vector.tensor_tensor(yt[:], yt[:], x_mid[:, j, :], ALU.add)
                nc.sync.dma_start(y_sh[j * P:(j + 1) * P, :], yt[:])

            # optional debug taps: copy internal DRAM buffers to outputs
            dbg_srcs = {
                "l_dram": l_dram,
                "ago_out": ago_out,
                "agg_out": agg_out,
            }
            for dname in debug_outs:
                src = dbg_srcs[dname]
                dt_out = nc.dram_tensor("dbg_" + dname, list(src.shape),
                                        src.dtype, kind="ExternalOutput")
                nc.sync.dma_start(dt_out[:], src[:])

    nc.compile()
    return nc


_CACHE = {}


def _ternarize(w):
    # per-tensor ternary absmean quantization (matches reference.weight_quant)
    mean_c = max(float(np.mean(np.abs(w), dtype=np.float64)), EPS)
    q = np.clip(np.round(w * np.float32(1.0 / mean_c)), -1, 1)
    return q.astype(np.float32), mean_c


def kernel(**inputs):
    m = _imports()
    import ml_dtypes
    bf16 = ml_dtypes.bfloat16

    x = np.ascontiguousarray(np.asarray(inputs["x"]), dtype=np.float32)
    assert int(inputs["num_heads"]) == H
    w_qkv = np.asarray(inputs["w_qkv"], np.float32)
    b_qkv = np.asarray(inputs["b_qkv"], np.float32)
    w_proj = np.asarray(inputs["w_proj"], np.float32)
    b_proj = np.asarray(inputs["b_proj"], np.float32)
    w_fc1 = np.asarray(inputs["w_fc1"], np.float32)
    b_fc1 = np.asarray(inputs["b_fc1"], np.float32)
    w_fc2 = np.asarray(inputs["w_fc2"], np.float32)
    b_fc2 = np.asarray(inputs["b_fc2"], np.float32)
    g1 = np.asarray(inputs["g1"], np.float32)
    be1 = np.asarray(inputs["be1"], np.float32)
    g2 = np.asarray(inputs["g2"], np.float32)
    be2 = np.asarray(inputs["be2"], np.float32)

    g1_trivial = bool(np.all(g1 == 1.0) and np.all(be1 == 0.0))
    g2_trivial = bool(np.all(g2 == 1.0) and np.all(be2 == 0.0))

    key = (g1_trivial, g2_trivial)
    if key not in _CACHE:
        _CACHE[key] = build_kernel(g1_trivial, g2_trivial)
    nc = _CACHE[key]

    qkv_q, mean_qkv = _ternarize(w_qkv)
    proj_q, mean_proj = _ternarize(w_proj)
    fc1_q, mean_fc1 = _ternarize(w_fc1)
    fc2_q, mean_fc2 = _ternarize(w_fc2)
    wsc127 = np.array([mean_qkv, mean_proj, mean_fc1, mean_fc2],
                      np.float32) / np.float32(127.0)

    in_maps = []
    for c in range(NCORES):
        g, r = divmod(c, TP)
        tok = slice(TOK * r, TOK * (r + 1))
        hsl = slice(CS * r, CS * (r + 1))
        im = {
            "x_sh": np.ascontiguousarray(x[g, tok]),
            "wqkT_q": np.ascontiguousarray(np.concatenate(
                [qkv_q[hsl, :].T, qkv_q[C:][hsl, :].T], axis=1)).astype(bf16),
            "wvT_q": np.ascontiguousarray(qkv_q[2 * C:][hsl, :].T).astype(bf16),
            "wpT_q": np.ascontiguousarray(proj_q.T).astype(bf16),
            "wf1T_q": np.ascontiguousarray(
                fc1_q[HS * r:HS * (r + 1), :].T).astype(bf16),
            "wf2T_q": np.ascontiguousarray(
                fc2_q[:, HS * r:HS * (r + 1)].T).astype(bf16),
            "wsc127": wsc127,
            "bqk": np.ascontiguousarray(
                np.concatenate([b_qkv[hsl], b_qkv[C:][hsl]])),
            "bv": np.ascontiguousarray(b_qkv[2 * C:][hsl]),
            "bp": b_proj,
            "onehot": np.eye(TP, dtype=np.float32)[r],
            "rank_in": np.array([[r]], dtype=np.uint32),
            "bf1": np.ascontiguousarray(b_fc1[HS * r:HS * (r + 1)]),
            "bf2": b_fc2,
        }
        if not g1_trivial:
            im["g1"], im["be1"] = g1, be1
        if not g2_trivial:
            im["g2"], im["be2"] = g2, be2
        in_maps.append(im)

    global _last_in_maps
    _last_in_maps = in_maps
    res = m["run"](nc, in_maps, core_ids=list(range(NCORES)))
    out = np.empty((B, N, C), np.float32)
    for c in range(NCORES):
        g, r = divmod(c, TP)
        out[g, TOK * r:TOK * (r + 1)] = res.results[c]["y_sh"]
    return out
